# revision 8
# baseline (speedup 1.0000x reference)
"""CRF negative-log-likelihood loss kernel for Trainium2 (Bass/Tile).

Problem: B=4096 sequences, L=4096 positions, T=2 tags, mask all-ones.
Reference: mean over batch of (logZ - gold_score) / L.

Strategy (pure data parallel, 8 cores, 512 sequences each):
  * Normalizer logZ via the transfer-matrix product
        P = M_{L-1} @ ... @ M_1 @ diag(exp(start + e_0)),
    M_l = diag(exp(e_l)) @ exp(Tr), computed as a balanced tree reduction
    in the *linear* (probability) domain up to 64-position blocks.  Every
    exp() carries a -1.0 bias so block entries stay ~exp(N(0, sigma)); the
    exact bias correction (+1.0 per position) is added back on the host.
    Blocks are converted to log domain (Ln) and combined with
    log-semiring 2x2 products (max + softplus) up the rest of the tree.
  * Level-0 pair combine uses the rank-1 structure
        (M_odd @ M_even)[i,j] = X_i * G1_ij * (r_ij * a + b)
    which is 2 fused scalar_tensor_tensor ops per matrix entry.
  * Gold score in closed form (T=2 => Tr[a,b] bilinear in a,b):
        gold = sum(e0) + sum(t*(e1-e0)) + cC*sum(t_l*t_{l-1})
             + cAB*sum(t) + ct0*t_0 + ctL*t_{L-1} + const
    computed with fused accumulating ops.
  * Work is split DVE / GPSIMD / ACT so no single engine is the wall.

The kernel is self-contained: shapes/sharding are hardcoded for the
4096x4096x2 problem; tags are narrowed to int32 host-side (values in
{0,1}); the mask is validated to be all-ones (guaranteed by the problem
spec) with a numpy fallback otherwise.
"""

import math
from contextlib import ExitStack

import numpy as np

import concourse.bass as bass
import concourse.tile as tile
from concourse import mybir
from concourse.bass_utils import run_bass_kernel_spmd

AF = mybir.ActivationFunctionType
OP = mybir.AluOpType
F32 = mybir.dt.float32
I32 = mybir.dt.int32

N_CORES = 8
P = 128          # SBUF partitions


def _ap(t, off, dims):
    """Custom AP on SBUF tile t: partition dim + given [step, count] dims."""
    base = t[:]
    return bass.AP(tensor=base.tensor, offset=base.offset + off,
                   ap=[base.ap[0]] + [list(d) for d in dims])


def _split_multiwaits(nc):
    """This container's walrus accepts only ONE sem wait per instruction;
    Tile's tail drain carries several.  Hoist extra waits onto same-engine
    single-wait drains inserted immediately before the instruction."""
    for f in nc.m.functions:
        for b in f.blocks:
            out = []
            changed = False
            for ins in b.instructions:
                si = ins.sync_info
                if si is not None and si.on_wait and len(si.on_wait) > 1:
                    waits = list(si.on_wait)
                    for k, w in enumerate(waits[:-1]):
                        d = mybir.InstDrain(name=f"{ins.name}-wsplit{k}")
                        d.engine = ins.engine
                        d.sync_info = mybir.SyncInfo(on_wait=[w], on_update=[])
                        out.append(d)
                    ins.sync_info = mybir.SyncInfo(
                        on_wait=[waits[-1]], on_update=list(si.on_update or []))
                    changed = True
                out.append(ins)
            if changed:
                b.instructions = out
    return nc


def _build(consts, G, L, C, BLK, debug=False):
    """Build the Bass program for one core: G groups of 128 sequences."""
    (K4, E4, goldc, cAB, cC, ct0, ctL, r_e, g1_e, CBIAS) = consts
    NCH = L // C          # chunks per group
    K1 = C // 2           # level-0 output matrices per chunk
    NLEV = int(math.log2(BLK)) - 1   # generic linear levels (1..NLEV)
    BPC = C // BLK        # blocks per chunk
    NBLK = L // BLK       # blocks per group
    ULEV = int(math.log2(NBLK))      # upper (log-domain) levels
    NCONST = 16

    nc = bass.Bass()
    em = nc.dram_tensor("emissions", [G * P, L, 2], F32, kind="ExternalInput")
    tg = nc.dram_tensor("tags", [G * P, L], I32, kind="ExternalInput")
    cst = nc.dram_tensor("consts", [1, NCONST], F32, kind="ExternalInput")
    nll = nc.dram_tensor("nll", [G, P], F32, kind="ExternalOutput")
    if debug:
        zdbg = nc.dram_tensor("zdbg", [G, P], F32, kind="ExternalOutput")
        gdbg = nc.dram_tensor("gdbg", [G, P], F32, kind="ExternalOutput")

    with tile.TileContext(nc) as tc, ExitStack() as ctx:
        io = ctx.enter_context(tc.tile_pool(name="io", bufs=3))
        wk = ctx.enter_context(tc.tile_pool(name="wk", bufs=2))
        ps = ctx.enter_context(tc.tile_pool(name="ps", bufs=1))

        # Persistent tiles
        CST = ps.tile([P, NCONST], F32, tag="cst")
        nc.sync.dma_start(out=CST, in_=bass.AP(
            tensor=cst[:].tensor, offset=0, ap=[[0, P], [1, NCONST]]))
        LOG = ps.tile([P, G * 4 * NBLK], F32, tag="log")       # block logs
        ACC = ps.tile([P, G * 4 * NCH], F32, tag="acc")        # gold accums
        TEF = ps.tile([P, 2 * G], F32, tag="tef")              # t0 / tLast
        BIASN = ps.tile([P, 1], F32, tag="biasn")              # -CBIAS for Exp
        nc.vector.memset(BIASN, -CBIAS)

        for g in range(G):
            for c in range(NCH):
                l0 = c * C
                # ---- loads ----
                E = io.tile([P, 2 * C], F32, tag="E")
                nc.sync.dma_start(out=_ap(E, 0, [[2, C], [1, 2]]),
                                  in_=em[g * P:(g + 1) * P, l0:l0 + C, :])
                ov = 0 if c == 0 else 1        # overlap 1 tag col for l-1
                TG = io.tile([P, C + 1], I32, tag="TG")
                nc.sync.dma_start(out=TG[:, :C + ov],
                                  in_=tg[g * P:(g + 1) * P, l0 - ov:l0 + C])

                # ---- exp(e - CBIAS)  (ACT) ----
                EX = wk.tile([P, 2 * C], F32, tag="EX")
                nc.scalar.activation(EX, E, AF.Exp, bias=BIASN[:, 0:1], scale=1.0)

                # ---- gold-score pieces ----
                D = wk.tile([P, C], F32, tag="D")         # e1 - e0
                nc.vector.tensor_tensor(out=D, in0=_ap(E, 1, [[2, C]]),
                                        in1=_ap(E, 0, [[2, C]]), op=OP.subtract)
                SCA = wk.tile([P, C], F32, tag="SCA")
                # sum e0 (ACT, fused accumulate)
                nc.scalar.activation(SCA, _ap(E, 0, [[2, C]]), AF.Copy,
                                     accum_out=ACC[:, (g * 4 + 0) * NCH + c:(g * 4 + 0) * NCH + c + 1])
                SCB = wk.tile([P, C], F32, tag="SCB")
                # sum t (ACT copy of int tags, fused accumulate)
                nc.scalar.activation(SCB, TG[:, ov:ov + C], AF.Copy,
                                     accum_out=ACC[:, (g * 4 + 2) * NCH + c:(g * 4 + 2) * NCH + c + 1])
                SCC = wk.tile([P, C], F32, tag="SCC")
                # sum t*(e1-e0)  (DVE stt with accumulate)
                nc.vector.scalar_tensor_tensor(
                    out=SCC, in0=TG[:, ov:ov + C], scalar=0.0, in1=D,
                    op0=OP.bypass, op1=OP.mult,
                    accum_out=ACC[:, (g * 4 + 1) * NCH + c:(g * 4 + 1) * NCH + c + 1])
                SCD = wk.tile([P, C], F32, tag="SCD")
                # sum t_l * t_{l-1} (covers chunk seam via overlap col)
                npair = C - 1 + ov
                nc.vector.scalar_tensor_tensor(
                    out=SCD[:, :npair], in0=TG[:, 1:1 + npair], scalar=0.0,
                    in1=TG[:, :npair], op0=OP.bypass, op1=OP.mult,
                    accum_out=ACC[:, (g * 4 + 3) * NCH + c:(g * 4 + 3) * NCH + c + 1])
                if c == 0:   # first tag
                    nc.scalar.activation(TEF[:, g:g + 1], TG[:, 0:1], AF.Copy)
                if c == NCH - 1:  # last tag
                    nc.scalar.activation(TEF[:, G + g:G + g + 1], TG[:, C + ov - 1:C + ov], AF.Copy)

                # ---- level 0: pair combine via rank-1 structure ----
                U = wk.tile([P, 4 * K1], F32, tag="U")
                C0 = wk.tile([P, 4 * K1], F32, tag="C0")
                for e in range(4):
                    i = e // 2
                    # u_e = r_e * a + b
                    nc.vector.scalar_tensor_tensor(
                        out=U[:, e * K1:(e + 1) * K1],
                        in0=_ap(EX, 0, [[4, K1]]), scalar=float(r_e[e]),
                        in1=_ap(EX, 1, [[4, K1]]), op0=OP.mult, op1=OP.add)
                    # C0_e = (u_e * g1_e) * X_i   (X_0 = c-hat, X_1 = d-hat)
                    nc.vector.scalar_tensor_tensor(
                        out=C0[:, e * K1:(e + 1) * K1],
                        in0=U[:, e * K1:(e + 1) * K1], scalar=float(g1_e[e]),
                        in1=_ap(EX, 2 + i, [[4, K1]]), op0=OP.mult, op1=OP.mult)
                if c == 0:
                    # patch k=0: C0[:,e*K1] = (exp(a0) column scale) form:
                    # R[i,j] = ehat1[i] * That[i,j]*shat[j] * ehat0[j]
                    P4 = wk.tile([P, 4], F32, tag="P4")
                    nc.vector.tensor_tensor(
                        out=P4, in0=_ap(EX, 2, [[1, 2], [0, 2]]),
                        in1=_ap(EX, 0, [[0, 2], [1, 2]]), op=OP.mult)
                    nc.vector.tensor_tensor(
                        out=_ap(C0, 0, [[K1, 4]]), in0=P4,
                        in1=CST[:, 0:4], op=OP.mult)

                # ---- generic linear levels (GPSIMD for level 1, DVE rest) ----
                prev, kp = C0, K1
                for v in range(1, NLEV + 1):
                    k = kp // 2
                    cur = wk.tile([P, 4 * k], F32, tag=f"L{v}")
                    tmp = wk.tile([P, 4 * k], F32, tag=f"T{v}")
                    eng = nc.gpsimd if v == 1 else nc.vector
                    out_ap = _ap(cur, 0, [[2 * k, 2], [k, 2], [1, k]])
                    tmp_ap = _ap(tmp, 0, [[2 * k, 2], [k, 2], [1, k]])
                    # A[i,mu] at e=2i+mu (odd m), B[mu,j] at e=2mu+j (even m)
                    a0 = _ap(prev, 0 * kp + 1, [[2 * kp, 2], [0, 2], [2, k]])
                    b0 = _ap(prev, 0 * kp + 0, [[0, 2], [kp, 2], [2, k]])
                    a1 = _ap(prev, 1 * kp + 1, [[2 * kp, 2], [0, 2], [2, k]])
                    b1 = _ap(prev, 2 * kp + 0, [[0, 2], [kp, 2], [2, k]])
                    eng.tensor_tensor(out=out_ap, in0=a0, in1=b0, op=OP.mult)
                    eng.tensor_tensor(out=tmp_ap, in0=a1, in1=b1, op=OP.mult)
                    eng.tensor_tensor(out=cur, in0=cur, in1=tmp, op=OP.add)
                    prev, kp = cur, k

                # ---- block log conversion ----
                nc.scalar.activation(
                    _ap(LOG, g * 4 * NBLK + c * BPC, [[NBLK, 4], [1, BPC]]),
                    prev, AF.Ln)

        # ---- upper tree in log domain, per group ----
        for g in range(G):
            prev_off, prev_t, kp = g * 4 * NBLK, LOG, NBLK
            for v in range(ULEV):
                k = kp // 2
                s0 = wk.tile([P, 4 * k], F32, tag=f"US0{v}")
                s1 = wk.tile([P, 4 * k], F32, tag=f"US1{v}")
                cur = wk.tile([P, 4 * k], F32, tag=f"UC{v}")
                oap = _ap(cur, 0, [[2 * k, 2], [k, 2], [1, k]])
                o1 = _ap(s1, 0, [[2 * k, 2], [k, 2], [1, k]])
                a0 = _ap(prev_t, prev_off + 0 * kp + 1, [[2 * kp, 2], [0, 2], [2, k]])
                b0 = _ap(prev_t, prev_off + 0 * kp + 0, [[0, 2], [kp, 2], [2, k]])
                a1 = _ap(prev_t, prev_off + 1 * kp + 1, [[2 * kp, 2], [0, 2], [2, k]])
                b1 = _ap(prev_t, prev_off + 2 * kp + 0, [[0, 2], [kp, 2], [2, k]])
                nc.vector.tensor_tensor(out=_ap(s0, 0, [[2 * k, 2], [k, 2], [1, k]]),
                                        in0=a0, in1=b0, op=OP.add)
                nc.vector.tensor_tensor(out=o1, in0=a1, in1=b1, op=OP.add)
                mx = wk.tile([P, 4 * k], F32, tag=f"UM{v}")
                mn = wk.tile([P, 4 * k], F32, tag=f"UN{v}")
                nc.vector.tensor_tensor(out=mx, in0=s0, in1=s1, op=OP.max)
                nc.vector.tensor_tensor(out=mn, in0=s0, in1=s1, op=OP.min)
                nc.vector.tensor_tensor(out=mn, in0=mn, in1=mx, op=OP.subtract)
                nc.scalar.activation(mn, mn, AF.Exp)
                nc.scalar.activation(mn, mn, AF.Ln, bias=1.0)  # ln(1+exp(d))
                nc.vector.tensor_tensor(out=cur, in0=mx, in1=mn, op=OP.add)
                prev_t, prev_off, kp = cur, 0, k

            # ---- finalize logZ for this group: lse over 4 entries + end ----
            ZT = wk.tile([P, 4], F32, tag="ZT")
            nc.vector.tensor_tensor(out=ZT, in0=prev_t, in1=CST[:, 4:8], op=OP.add)
            ZM = wk.tile([P, 1], F32, tag="ZM")
            nc.vector.tensor_reduce(out=ZM, in_=ZT, axis=mybir.AxisListType.X,
                                    op=OP.max)
            ZS = wk.tile([P, 4], F32, tag="ZS")
            nc.vector.tensor_tensor(out=ZS, in0=ZT,
                                    in1=_ap(ZM, 0, [[0, 4]]), op=OP.subtract)
            nc.scalar.activation(ZS, ZS, AF.Exp)
            ZP = wk.tile([P, 1], F32, tag="ZP")
            nc.vector.tensor_reduce(out=ZP, in_=ZS, axis=mybir.AxisListType.X,
                                    op=OP.add)
            nc.scalar.activation(ZP, ZP, AF.Ln)
            Z = wk.tile([P, 1], F32, tag="Z")
            nc.vector.tensor_tensor(out=Z, in0=ZP, in1=ZM, op=OP.add)

            # ---- gold score for this group ----
            SM = wk.tile([P, 4], F32, tag="SM")
            nc.vector.tensor_reduce(
                out=SM, in_=_ap(ACC, g * 4 * NCH, [[NCH, 4], [1, NCH]]),
                axis=mybir.AxisListType.X, op=OP.add)
            G1 = wk.tile([P, 1], F32, tag="G1")
            # cAB*sum(t) + sum(e0)
            nc.vector.scalar_tensor_tensor(out=G1, in0=SM[:, 2:3], scalar=cAB,
                                           in1=SM[:, 0:1], op0=OP.mult, op1=OP.add)
            G2 = wk.tile([P, 1], F32, tag="G2")
            # cC*sum(tt) + sum(t*d)
            nc.vector.scalar_tensor_tensor(out=G2, in0=SM[:, 3:4], scalar=cC,
                                           in1=SM[:, 1:2], op0=OP.mult, op1=OP.add)
            nc.vector.tensor_tensor(out=G1, in0=G1, in1=G2, op=OP.add)
            nc.vector.scalar_tensor_tensor(out=G1, in0=TEF[:, g:g + 1], scalar=ct0,
                                           in1=G1, op0=OP.mult, op1=OP.add)
            nc.vector.scalar_tensor_tensor(out=G1, in0=TEF[:, G + g:G + g + 1],
                                           scalar=ctL, in1=G1,
                                           op0=OP.mult, op1=OP.add)
            nc.vector.tensor_scalar(out=G1, in0=G1, scalar1=goldc, scalar2=None,
                                    op0=OP.add)
            # nll = (Z - gold) / L
            NL = wk.tile([P, 1], F32, tag="NL")
            nc.vector.tensor_tensor(out=NL, in0=Z, in1=G1, op=OP.subtract)
            nc.vector.tensor_scalar(out=NL, in0=NL, scalar1=1.0 / L, scalar2=None,
                                    op0=OP.mult)
            nc.sync.dma_start(out=nll[g:g + 1, :], in_=NL)
            if debug:
                nc.sync.dma_start(out=zdbg[g:g + 1, :], in_=Z)
                nc.sync.dma_start(out=gdbg[g:g + 1, :], in_=G1)

    return _split_multiwaits(nc)


_CACHE = {}
LAST_RESULTS = None


def _get_nc(key, consts, G, L, C, BLK):
    if key not in _CACHE:
        _CACHE[key] = _build(consts, G, L, C, BLK)
    return _CACHE[key]


def _host_consts(transitions, start_transitions, end_transitions, L, CBIAS=1.0):
    tr = np.asarray(transitions, np.float64)
    st = np.asarray(start_transitions, np.float64)
    en = np.asarray(end_transitions, np.float64)
    Th = np.exp(tr)
    sh = np.exp(st)
    K4 = np.array([Th[i, j] * sh[j] for i in (0, 1) for j in (0, 1)], np.float64)
    E4 = np.array([en[0], en[0], en[1], en[1]], np.float64)
    A = tr[1, 0] - tr[0, 0]
    Bc = tr[0, 1] - tr[0, 0]
    cC = tr[1, 1] - tr[1, 0] - tr[0, 1] + tr[0, 0]
    goldc = (L - 1) * tr[0, 0] + st[0] + en[0]
    cAB = A + Bc
    ct0 = st[1] - st[0] - A
    ctL = en[1] - en[0] - Bc
    G0 = np.array([Th[i, 0] * Th[0, j] for i in (0, 1) for j in (0, 1)])
    G1 = np.array([Th[i, 1] * Th[1, j] for i in (0, 1) for j in (0, 1)])
    r_e = G0 / G1
    return (tuple(np.float32(K4)), tuple(np.float32(E4)), float(np.float32(goldc)),
            float(np.float32(cAB)), float(np.float32(cC)), float(np.float32(ct0)),
            float(np.float32(ctL)), tuple(np.float32(r_e)), tuple(np.float32(G1)),
            float(CBIAS))


def _np_crf_fallback(emissions, tags, mask, transitions, start_transitions,
                     end_transitions):
    """Plain numpy CRF NLL (general mask) — correctness fallback only."""
    em = np.asarray(emissions, np.float64)
    tg = np.asarray(tags, np.int64)
    mk = np.asarray(mask, bool)
    tr = np.asarray(transitions, np.float64)
    st = np.asarray(start_transitions, np.float64)
    en = np.asarray(end_transitions, np.float64)
    B, L, T = em.shape
    score = st[tg[:, 0]] + em[np.arange(B), 0, tg[:, 0]]
    for l in range(1, L):
        emit = em[np.arange(B), l, tg[:, l]]
        trans = tr[tg[:, l], tg[:, l - 1]]
        score += (emit + trans) * mk[:, l]
    alpha = st[None, :] + em[:, 0]
    for l in range(1, L):
        sc = alpha[:, None, :] + tr[None, :, :]
        m = sc.max(axis=2, keepdims=True)
        a_new = np.log(np.exp(sc - m).sum(axis=2)) + m[:, :, 0] + em[:, l]
        alpha = np.where(mk[:, l, None], a_new, alpha)
    m = (alpha + en).max(axis=1, keepdims=True)
    logz = np.log(np.exp(alpha + en - m).sum(axis=1)) + m[:, 0]
    sl = np.maximum(mk.sum(axis=1), 1.0)
    return np.float32(((logz - score) / sl).mean())


def kernel(emissions, tags, mask, transitions, start_transitions,
           end_transitions):
    B, L, T = emissions.shape
    assert T == 2
    if not np.all(mask):
        return _np_crf_fallback(emissions, tags, mask, transitions,
                                start_transitions, end_transitions)

    CBIAS = 1.0
    consts = _host_consts(transitions, start_transitions, end_transitions, L,
                          CBIAS)
    BS = B // N_CORES
    G = BS // P
    C = 1024 if L % 1024 == 0 else L
    BLK = 64
    key = (consts, G, L, C, BLK)
    nc = _get_nc(key, consts, G, L, C, BLK)

    em = np.ascontiguousarray(emissions, dtype=np.float32)
    tg = np.ascontiguousarray(tags, dtype=np.int32)
    NCONST = 16
    cvec = np.zeros((1, NCONST), np.float32)
    (K4, E4, goldc, cAB, cC, ct0, ctL, r_e, g1_e, _) = consts
    cvec[0, 0:4] = K4
    cvec[0, 4:8] = E4

    in_maps = []
    for c in range(N_CORES):
        in_maps.append({
            "emissions": em[c * BS:(c + 1) * BS],
            "tags": tg[c * BS:(c + 1) * BS],
            "consts": cvec,
        })
    global LAST_RESULTS
    res = run_bass_kernel_spmd(nc, in_maps, core_ids=list(range(N_CORES)))
    LAST_RESULTS = res
    nlls = np.concatenate([r["nll"].reshape(-1) for r in res.results])
    return np.float32(np.mean(nlls, dtype=np.float64) + CBIAS)
